# revision 10
# baseline (speedup 1.0000x reference)
"""LIF neuron kernel for Trainium2 (Bass/Tile), 8-core SPMD, bf16 streaming.

Reference computation (per problem nn_LIF_69707319214329):
    v_new      = v * DECAY + sum(x, axis=1) * 10         # [IN]
    fired      = v_new >= THRESHOLD                      # [IN]
    spikes_new = where(fired, 1.0, spikes)               # [IN]
    out        = spikes_new[None, :] * weight            # [OUT, IN]

Sharding: in_features (columns of weight / rows of x) are split into 8
contiguous blocks of 1024.  Core j receives x rows [1024j, 1024j+1024),
the matching v/spikes slices, and weight[:, block] (made contiguous on the
host).  Each core computes its own spikes slice locally -- no collectives --
and produces out[:, block].

Precision: the correctness gate is rel_err < 2e-2 while bf16 quantization
error is <= 2^-9 ~= 0.2%, so the weight is streamed in bf16 and the output
is produced in bf16 (upcast to fp32 on the host).  spikes_new is exactly
{0.0, 1.0}, so the multiply adds no error beyond the bf16 rounding of
weight.  Per-core HBM traffic: 2MB x + 16MB weight read + 16MB output
write = 34MB (vs 68MB all-fp32), against the ~358 GB/s HBM-per-core limit.

Engine plan: weight loads stream on the sync HWDGE ring, output stores on
the scalar HWDGE ring (one x tile on each ring first).  All small phase-1
DMAs (v/s loads, spike-row flatten) go through gpsimd SWDGE so neither
HWDGE sequencer ever stalls behind a compute dependency.
"""

import math

import numpy as np
import ml_dtypes

import concourse.bass as bass
import concourse.bacc as bacc
import concourse.mybir as mybir
from concourse.tile import TileContext
from concourse.bass_utils import run_bass_kernel_spmd

N_CORES = 8
IN_FEATURES = 8192
OUT_FEATURES = 8192
K = 1024
SHARD = IN_FEATURES // N_CORES          # 1024 in_features per core
TAU = 1.0
THRESHOLD = 20.0
DECAY = math.exp(-0.01 / TAU)

F32 = mybir.dt.float32
BF16 = mybir.dt.bfloat16
NP_BF16 = ml_dtypes.bfloat16
W_BYTES = 2                             # weight/output stream dtype size
X_BYTES = 2                             # x stream dtype size

# Main-loop tiling: weight shard [8192, 1024] bf16 seen as ROW_TILES tiles of
# [128, ROWS_PER_PART * 1024]; partition p of tile r holds weight rows
# r*ROWS_PER_TILE + p*ROWS_PER_PART ... + ROWS_PER_PART-1 (contiguous bytes).
# 1MB tiles (rpp=4): same steady-state rate as 2MB in sim, shorter ramp/drain.
ROWS_PER_PART = 4
ROWS_PER_TILE = 128 * ROWS_PER_PART     # 1024
ROW_TILES = OUT_FEATURES // ROWS_PER_TILE

# x shard [1024, 1024] bf16 loaded as X_TILES tiles of
# [128, X_ROWS_PER_PART*1024].  The host pre-permutes x rows (and v/spikes)
# so that the LIF state tile ends up as rs[p, c] = local in_feature 8p + c --
# then flattening spk [128, 8] to the spikes row [1, 1024] is the identity
# (p, c) iteration, a plain contiguous SBUF->SBUF DMA with no transpose.
X_ROWS_PER_PART = 4
X_TILES = SHARD // (128 * X_ROWS_PER_PART)  # 2 x-tiles, one per HWDGE ring
T_COLS = SHARD // 128                   # 8 state columns

# host permutation: x_perm[j] = x[PERM[j]]; load AP puts perm row
# 512t + 128a + p on partition p, state column c = 4t + a, and we need
# state (p, c) == original in_feature 8p + c.
_J = np.arange(SHARD)
PERM = 8 * (_J % 128) + _J // 128


def _build_bass(
    reps: int = 1,
    rows_per_part: int = ROWS_PER_PART,
    wbufs: int = 8,
    fake_spikes: bool = False,
) -> bass.Bass:
    """reps>1 repeats the phase-2 weight stream (for HW timing via deltas);
    output is identical since every pass writes the same values."""
    pattern = [rows_per_part] * (OUT_FEATURES // (128 * rows_per_part))
    assert sum(pattern) * 128 == OUT_FEATURES
    segments = []          # (row_start, rows_per_part)
    row0 = 0
    for rpp in pattern:
        segments.append((row0, rpp))
        row0 += 128 * rpp

    nc = bacc.Bacc(
        "TRN2",
        target_bir_lowering=False,
        debug=False,
        num_devices=N_CORES,
    )

    x = nc.dram_tensor("x", [SHARD, K], BF16, kind="ExternalInput")
    w = nc.dram_tensor("w", [OUT_FEATURES, SHARD], BF16, kind="ExternalInput")
    v = nc.dram_tensor("v", [128, T_COLS], F32, kind="ExternalInput")
    s = nc.dram_tensor("s", [128, T_COLS], F32, kind="ExternalInput")
    o = nc.dram_tensor("o", [OUT_FEATURES, SHARD], BF16, kind="ExternalOutput")

    with TileContext(nc) as tc:
        with (
            tc.tile_pool(name="state", bufs=1) as state,
            tc.tile_pool(name="xp", bufs=2) as xp,
            tc.tile_pool(name="wp", bufs=wbufs) as wp,
        ):
            # ---- Phase 1: LIF state -> broadcast spike row ----
            if fake_spikes:
                # timing-only variant: skip the LIF computation to isolate
                # phase-1's contribution to the sim/HW overhead
                bb = state.tile([128, SHARD], BF16)
                nc.vector.memset(bb[:], 1.0)

            rs = state.tile([128, T_COLS], F32)
            for t in range(X_TILES) if not fake_spikes else []:
                xt = xp.tile([128, X_ROWS_PER_PART, K], BF16)
                # rows a*128 + p for a in range(X_ROWS_PER_PART)
                src = x[t * 128 * X_ROWS_PER_PART:(t + 1) * 128 * X_ROWS_PER_PART, :]
                src = src.rearrange("(a p) c -> p a c", p=128)
                # one x tile on each HWDGE ring, ahead of the weight stream
                dma_eng = nc.sync if t % 2 == 0 else nc.scalar
                dma_eng.dma_start(out=xt[:], in_=src)
                nc.vector.reduce_sum(
                    out=rs[:, t * X_ROWS_PER_PART:(t + 1) * X_ROWS_PER_PART],
                    in_=xt[:],
                    axis=mybir.AxisListType.X,
                )

            if not fake_spikes:
                vt = state.tile([128, T_COLS], F32)
                st = state.tile([128, T_COLS], F32)
                nc.gpsimd.dma_start(out=vt[:], in_=v[:])
                nc.gpsimd.dma_start(out=st[:], in_=s[:])

                # Everything that depends only on v/s runs while x streams:
                # fired = (v*DECAY + rs*10 >= THR)  <=>  rs >= (THR - v*DECAY)/10
                thr = state.tile([128, T_COLS], F32)
                nc.vector.tensor_scalar(
                    out=thr[:],
                    in0=vt[:],
                    scalar1=-DECAY / 10.0,
                    scalar2=THRESHOLD / 10.0,
                    op0=mybir.AluOpType.mult,
                    op1=mybir.AluOpType.add,
                )
                ones = state.tile([128, T_COLS], F32)
                nc.vector.memset(ones[:], 1.0)
                spk = state.tile([128, T_COLS], F32)
                nc.vector.tensor_copy(out=spk[:], in_=st[:])

                # critical path after the last x reduce: compare + select
                mask = state.tile([128, T_COLS], mybir.dt.uint32)
                nc.vector.tensor_tensor(
                    out=mask[:], in0=rs[:], in1=thr[:], op=mybir.AluOpType.is_ge
                )
                nc.vector.copy_predicated(spk[:], mask[:], ones[:])

                # flatten spk [128, T_COLS] -> bf16 row [1, SHARD], casting in
                # the SWDGE DMA.  Thanks to the host permutation this is the
                # identity iteration order (128 x 32B descriptors).  SWDGE so
                # the HWDGE rings never stall on this compute-dependent DMA.
                rowh = state.tile([1, SHARD], BF16)
                nc.gpsimd.dma_start(out=rowh[:1, :], in_=spk[:])
                bb = state.tile([128, SHARD], BF16)
                nc.gpsimd.partition_broadcast(bb[:], rowh[:1, :])

            bb_row = bb[:, :].rearrange("p (z c) -> p z c", z=1)
            bb_bcast = {
                rpp: bb_row.broadcast_to([128, rpp, SHARD])
                for rpp in set(pattern)
            }

            # ---- Phase 2: out = weight * spikes (column-broadcast) ----
            for row0, rpp in (sg for _ in range(reps) for sg in segments):
                nrows = 128 * rpp
                wt = wp.tile([128, rpp * SHARD], BF16, tag="wt")
                src = w[row0:row0 + nrows, :]
                src = src.rearrange("(p a) c -> p (a c)", a=rpp)
                nc.sync.dma_start(out=wt[:], in_=src)

                nc.vector.tensor_mul(
                    out=wt[:].rearrange("p (a c) -> p a c", a=rpp),
                    in0=wt[:].rearrange("p (a c) -> p a c", a=rpp),
                    in1=bb_bcast[rpp],
                )

                dst = o[row0:row0 + nrows, :]
                dst = dst.rearrange("(p a) c -> p (a c)", a=rpp)
                nc.scalar.dma_start(out=dst, in_=wt[:])

    nc.compile()
    return nc


_NC_CACHE = {}


def _get_bass(reps: int = 1, **kwargs) -> bass.Bass:
    key = (reps, tuple(sorted(kwargs.items())))
    if key not in _NC_CACHE:
        _NC_CACHE[key] = _build_bass(reps, **kwargs)
    return _NC_CACHE[key]


def _shard_inputs(x, weight, v, spikes):
    x16 = x.astype(NP_BF16)
    w16 = weight.astype(NP_BF16)
    in_maps = []
    for j in range(N_CORES):
        sl = slice(j * SHARD, (j + 1) * SHARD)
        in_maps.append({
            "x": np.ascontiguousarray(x16[sl, :][PERM]),
            "w": np.ascontiguousarray(w16[:, sl]),
            "v": np.ascontiguousarray(v[sl].reshape(128, T_COLS)),
            "s": np.ascontiguousarray(spikes[sl].reshape(128, T_COLS)),
        })
    return in_maps


def run(x, weight, v, spikes, trace=False, **run_kwargs):
    """Run the 8-core kernel; returns (full_output, BassKernelResults)."""
    x = np.asarray(x, dtype=np.float32)
    weight = np.asarray(weight, dtype=np.float32)
    v = np.asarray(v, dtype=np.float32)
    spikes = np.asarray(spikes, dtype=np.float32)
    assert x.shape == (IN_FEATURES, K)
    assert weight.shape == (OUT_FEATURES, IN_FEATURES)

    nc = _get_bass()
    in_maps = _shard_inputs(x, weight, v, spikes)
    res = run_bass_kernel_spmd(
        nc, in_maps, core_ids=list(range(N_CORES)), trace=trace, **run_kwargs
    )
    out = np.empty((OUT_FEATURES, IN_FEATURES), dtype=np.float32)
    for j in range(N_CORES):
        out[:, j * SHARD:(j + 1) * SHARD] = res.results[j]["o"].astype(np.float32)
    return out, res


def kernel(x, weight, v, spikes, t=None, **_ignored):
    out, _ = run(x, weight, v, spikes, trace=False)
    return out


# revision 14
# speedup vs baseline: 1.1452x; 1.1452x over previous
"""LIF neuron kernel for Trainium2 (Bass/Tile), 8-core SPMD, bf16 streaming.

Reference computation (per problem nn_LIF_69707319214329):
    v_new      = v * DECAY + sum(x, axis=1) * 10         # [IN]
    fired      = v_new >= THRESHOLD                      # [IN]
    spikes_new = where(fired, 1.0, spikes)               # [IN]
    out        = spikes_new[None, :] * weight            # [OUT, IN]

Sharding: in_features (columns of weight / rows of x) are split into 8
contiguous blocks of 1024.  Core j receives x rows [1024j, 1024j+1024),
the matching v/spikes slices, and weight[:, block] (made contiguous on the
host).  Each core computes its own spikes slice locally -- no collectives --
and produces out[:, block].

Precision: the correctness gate is rel_err < 2e-2 while bf16 quantization
error is <= 2^-9 ~= 0.2%, so the weight is streamed in bf16 and the output
is produced in bf16 (upcast to fp32 on the host).  spikes_new is exactly
{0.0, 1.0}, so the multiply adds no error beyond the bf16 rounding of
weight.  Per-core HBM traffic: 2MB x + 16MB weight read + 16MB output
write = 34MB (vs 68MB all-fp32), against the ~358 GB/s HBM-per-core limit.

Engine plan: weight loads stream on the sync HWDGE ring, output stores on
the scalar HWDGE ring (one x tile on each ring first).  All small phase-1
DMAs (v/s loads, spike-row flatten) go through gpsimd SWDGE so neither
HWDGE sequencer ever stalls behind a compute dependency.
"""

import math

import numpy as np
import ml_dtypes

import concourse.bass as bass
import concourse.bacc as bacc
import concourse.mybir as mybir
from concourse.tile import TileContext
from concourse.bass_utils import run_bass_kernel_spmd

N_CORES = 8
IN_FEATURES = 8192
OUT_FEATURES = 8192
K = 1024
SHARD = IN_FEATURES // N_CORES          # 1024 in_features per core
TAU = 1.0
THRESHOLD = 20.0
DECAY = math.exp(-0.01 / TAU)

F32 = mybir.dt.float32
BF16 = mybir.dt.bfloat16
NP_BF16 = ml_dtypes.bfloat16
W_BYTES = 2                             # weight/output stream dtype size
X_BYTES = 2                             # x stream dtype size

# Main-loop tiling: weight shard [8192, 1024] bf16 seen as ROW_TILES tiles of
# [128, ROWS_PER_PART * 1024]; partition p of tile r holds weight rows
# r*ROWS_PER_TILE + p*ROWS_PER_PART ... + ROWS_PER_PART-1 (contiguous bytes).
# 1MB tiles (rpp=4): same steady-state rate as 2MB in sim, shorter ramp/drain.
ROWS_PER_PART = 4
ROWS_PER_TILE = 128 * ROWS_PER_PART     # 1024
ROW_TILES = OUT_FEATURES // ROWS_PER_TILE

# x shard [1024, 1024] bf16 loaded as X_TILES tiles of
# [128, X_ROWS_PER_PART*1024].  The host pre-permutes x rows (and v/spikes)
# so that the LIF state tile ends up as rs[p, c] = local in_feature 8p + c --
# then flattening spk [128, 8] to the spikes row [1, 1024] is the identity
# (p, c) iteration, a plain contiguous SBUF->SBUF DMA with no transpose.
X_ROWS_PER_PART = 4
X_TILES = SHARD // (128 * X_ROWS_PER_PART)  # 2 x-tiles, one per HWDGE ring
T_COLS = SHARD // 128                   # 8 state columns

# host permutation: x_perm[j] = x[PERM[j]]; load AP puts perm row
# 512t + 128a + p on partition p, state column c = 4t + a, and we need
# state (p, c) == original in_feature 8p + c.
_J = np.arange(SHARD)
PERM = 8 * (_J % 128) + _J // 128


def _build_bass(
    reps: int = 1,
    rows_per_part: int = ROWS_PER_PART,
    wbufs: int = 8,
    fake_spikes: bool = False,
    loop_reps: int = 0,
    loop_unroll: int = 8,
    graded: bool = False,
) -> bass.Bass:
    """reps>1 repeats the phase-2 weight stream (for HW timing via deltas);
    output is identical since every pass writes the same values.

    loop_reps>0 instead wraps `loop_unroll` unrolled passes in a tc.For_i
    hardware loop executed loop_reps times (loop_reps*loop_unroll passes
    total at constant instruction count) -- the robust HW timing variant:
    per-NEFF device time can be pushed to ~100ms so axon dispatch jitter
    (+-tens of ms) divides out, while the NEFF stays small enough that
    instruction fetch never leaves IRAM."""
    if graded:
        # small head/tail tiles: first store starts ~4x sooner, last-store
        # drain ~4x shorter; middle tiles carry the stream
        n_mid = (OUT_FEATURES // 128 - 8) // rows_per_part
        pattern = [1, 1, 2] + [rows_per_part] * n_mid + [2, 1, 1]
    else:
        pattern = [rows_per_part] * (OUT_FEATURES // (128 * rows_per_part))
    assert sum(pattern) * 128 == OUT_FEATURES
    segments = []          # (row_start, rows_per_part)
    row0 = 0
    for rpp in pattern:
        segments.append((row0, rpp))
        row0 += 128 * rpp

    nc = bacc.Bacc(
        "TRN2",
        target_bir_lowering=False,
        debug=False,
        num_devices=N_CORES,
    )

    x = nc.dram_tensor("x", [SHARD, K], BF16, kind="ExternalInput")
    w = nc.dram_tensor("w", [OUT_FEATURES, SHARD], BF16, kind="ExternalInput")
    v = nc.dram_tensor("v", [128, T_COLS], F32, kind="ExternalInput")
    s = nc.dram_tensor("s", [128, T_COLS], F32, kind="ExternalInput")
    o = nc.dram_tensor("o", [OUT_FEATURES, SHARD], BF16, kind="ExternalOutput")

    with TileContext(nc) as tc:
        with (
            tc.tile_pool(name="state", bufs=1) as state,
            tc.tile_pool(name="xp", bufs=2) as xp,
            tc.tile_pool(name="wp", bufs=wbufs) as wp,
        ):
            # ---- Phase 1: LIF state -> broadcast spike row ----
            if fake_spikes:
                # timing-only variant: skip the LIF computation to isolate
                # phase-1's contribution to the sim/HW overhead
                bb = state.tile([128, SHARD], BF16)
                nc.vector.memset(bb[:], 1.0)

            rs = state.tile([128, T_COLS], F32)
            for t in range(X_TILES) if not fake_spikes else []:
                xt = xp.tile([128, X_ROWS_PER_PART, K], BF16)
                # rows a*128 + p for a in range(X_ROWS_PER_PART)
                src = x[t * 128 * X_ROWS_PER_PART:(t + 1) * 128 * X_ROWS_PER_PART, :]
                src = src.rearrange("(a p) c -> p a c", p=128)
                # one x tile on each HWDGE ring, ahead of the weight stream
                dma_eng = nc.sync if t % 2 == 0 else nc.scalar
                dma_eng.dma_start(out=xt[:], in_=src)
                nc.vector.reduce_sum(
                    out=rs[:, t * X_ROWS_PER_PART:(t + 1) * X_ROWS_PER_PART],
                    in_=xt[:],
                    axis=mybir.AxisListType.X,
                )

            if not fake_spikes:
                vt = state.tile([128, T_COLS], F32)
                st = state.tile([128, T_COLS], F32)
                nc.gpsimd.dma_start(out=vt[:], in_=v[:])
                nc.gpsimd.dma_start(out=st[:], in_=s[:])

                # Everything that depends only on v/s runs while x streams:
                # fired = (v*DECAY + rs*10 >= THR)  <=>  rs >= (THR - v*DECAY)/10
                thr = state.tile([128, T_COLS], F32)
                nc.vector.tensor_scalar(
                    out=thr[:],
                    in0=vt[:],
                    scalar1=-DECAY / 10.0,
                    scalar2=THRESHOLD / 10.0,
                    op0=mybir.AluOpType.mult,
                    op1=mybir.AluOpType.add,
                )
                ones = state.tile([128, T_COLS], F32)
                nc.vector.memset(ones[:], 1.0)
                spk = state.tile([128, T_COLS], F32)
                nc.vector.tensor_copy(out=spk[:], in_=st[:])

                # critical path after the last x reduce: compare + select
                mask = state.tile([128, T_COLS], mybir.dt.uint32)
                nc.vector.tensor_tensor(
                    out=mask[:], in0=rs[:], in1=thr[:], op=mybir.AluOpType.is_ge
                )
                nc.vector.copy_predicated(spk[:], mask[:], ones[:])

                # flatten spk [128, T_COLS] -> bf16 row [1, SHARD], casting in
                # the SWDGE DMA.  Thanks to the host permutation this is the
                # identity iteration order (128 x 32B descriptors).  SWDGE so
                # the HWDGE rings never stall on this compute-dependent DMA.
                rowh = state.tile([1, SHARD], BF16)
                nc.gpsimd.dma_start(out=rowh[:1, :], in_=spk[:])
                bb = state.tile([128, SHARD], BF16)
                nc.gpsimd.partition_broadcast(bb[:], rowh[:1, :])

            bb_row = bb[:, :].rearrange("p (z c) -> p z c", z=1)
            bb_bcast = {
                rpp: bb_row.broadcast_to([128, rpp, SHARD])
                for rpp in set(pattern)
            }

            # ---- Phase 2: out = weight * spikes (column-broadcast) ----
            def emit_pass(sgs):
                for row0, rpp in sgs:
                    nrows = 128 * rpp
                    wt = wp.tile([128, rpp * SHARD], BF16, tag="wt")
                    src = w[row0:row0 + nrows, :]
                    src = src.rearrange("(p a) c -> p (a c)", a=rpp)
                    nc.sync.dma_start(out=wt[:], in_=src)

                    nc.vector.tensor_mul(
                        out=wt[:].rearrange("p (a c) -> p a c", a=rpp),
                        in0=wt[:].rearrange("p (a c) -> p a c", a=rpp),
                        in1=bb_bcast[rpp],
                    )

                    dst = o[row0:row0 + nrows, :]
                    dst = dst.rearrange("(p a) c -> p (a c)", a=rpp)
                    nc.scalar.dma_start(out=dst, in_=wt[:])

            if loop_reps > 0:
                with tc.For_i(0, loop_reps, 1):
                    emit_pass(sg for _ in range(loop_unroll) for sg in segments)
            else:
                emit_pass(sg for _ in range(reps) for sg in segments)

    nc.compile()
    return nc


_NC_CACHE = {}


def _get_bass(reps: int = 1, **kwargs) -> bass.Bass:
    key = (reps, tuple(sorted(kwargs.items())))
    if key not in _NC_CACHE:
        _NC_CACHE[key] = _build_bass(reps, **kwargs)
    return _NC_CACHE[key]


def _shard_inputs(x, weight, v, spikes):
    x16 = x.astype(NP_BF16)
    w16 = weight.astype(NP_BF16)
    in_maps = []
    for j in range(N_CORES):
        sl = slice(j * SHARD, (j + 1) * SHARD)
        in_maps.append({
            "x": np.ascontiguousarray(x16[sl, :][PERM]),
            "w": np.ascontiguousarray(w16[:, sl]),
            "v": np.ascontiguousarray(v[sl].reshape(128, T_COLS)),
            "s": np.ascontiguousarray(spikes[sl].reshape(128, T_COLS)),
        })
    return in_maps


def run(x, weight, v, spikes, trace=False, **run_kwargs):
    """Run the 8-core kernel; returns (full_output, BassKernelResults)."""
    x = np.asarray(x, dtype=np.float32)
    weight = np.asarray(weight, dtype=np.float32)
    v = np.asarray(v, dtype=np.float32)
    spikes = np.asarray(spikes, dtype=np.float32)
    assert x.shape == (IN_FEATURES, K)
    assert weight.shape == (OUT_FEATURES, IN_FEATURES)

    nc = _get_bass()
    in_maps = _shard_inputs(x, weight, v, spikes)
    res = run_bass_kernel_spmd(
        nc, in_maps, core_ids=list(range(N_CORES)), trace=trace, **run_kwargs
    )
    out = np.empty((OUT_FEATURES, IN_FEATURES), dtype=np.float32)
    for j in range(N_CORES):
        out[:, j * SHARD:(j + 1) * SHARD] = res.results[j]["o"].astype(np.float32)
    return out, res


def kernel(x, weight, v, spikes, t=None, **_ignored):
    out, _ = run(x, weight, v, spikes, trace=False)
    return out


# revision 24
# speedup vs baseline: 1.1865x; 1.0360x over previous
"""LIF neuron kernel for Trainium2 (Bass/Tile), 8-core SPMD, bf16 streaming.

Reference computation (per problem nn_LIF_69707319214329):
    v_new      = v * DECAY + sum(x, axis=1) * 10         # [IN]
    fired      = v_new >= THRESHOLD                      # [IN]
    spikes_new = where(fired, 1.0, spikes)               # [IN]
    out        = spikes_new[None, :] * weight            # [OUT, IN]

Sharding: in_features (columns of weight / rows of x) are split into 8
contiguous blocks of 1024.  Core j receives x rows [1024j, 1024j+1024),
the matching v/spikes slices, and weight[:, block] (made contiguous on the
host).  Each core computes its own spikes slice locally -- no collectives --
and produces out[:, block].

Precision: the correctness gate is rel_err < 2e-2 while bf16 quantization
error is <= 2^-9 ~= 0.2%, so the weight is streamed in bf16 and the output
is produced in bf16 (upcast to fp32 on the host).  spikes_new is exactly
{0.0, 1.0}, so the multiply adds no error beyond the bf16 rounding of
weight.  Per-core HBM traffic: 2MB x + 16MB weight read + 16MB output
write = 34MB (vs 68MB all-fp32), against the ~358 GB/s HBM-per-core limit.

Engine plan: weight loads stream on the sync HWDGE ring, output stores on
the scalar HWDGE ring (one x tile on each ring first).  All small phase-1
DMAs (v/s loads, spike-row flatten) go through gpsimd SWDGE so neither
HWDGE sequencer ever stalls behind a compute dependency.
"""

import math

import numpy as np
import ml_dtypes

import concourse.bass as bass
import concourse.bacc as bacc
import concourse.mybir as mybir
from concourse.tile import TileContext
from concourse.bass_utils import run_bass_kernel_spmd

N_CORES = 8
IN_FEATURES = 8192
OUT_FEATURES = 8192
K = 1024
SHARD = IN_FEATURES // N_CORES          # 1024 in_features per core
TAU = 1.0
THRESHOLD = 20.0
DECAY = math.exp(-0.01 / TAU)

F32 = mybir.dt.float32
BF16 = mybir.dt.bfloat16
NP_BF16 = ml_dtypes.bfloat16
W_BYTES = 2                             # weight/output stream dtype size
X_BYTES = 2                             # x stream dtype size

# Main-loop tiling: weight shard [8192, 1024] bf16 seen as ROW_TILES tiles of
# [128, ROWS_PER_PART * 1024]; partition p of tile r holds weight rows
# r*ROWS_PER_TILE + p*ROWS_PER_PART ... + ROWS_PER_PART-1 (contiguous bytes).
# 1MB tiles (rpp=4): same steady-state rate as 2MB in sim, shorter ramp/drain.
ROWS_PER_PART = 4
ROWS_PER_TILE = 128 * ROWS_PER_PART     # 1024
ROW_TILES = OUT_FEATURES // ROWS_PER_TILE

# x shard [1024, 1024] bf16 loaded as X_TILES tiles of
# [128, X_ROWS_PER_PART*1024].  The host pre-permutes x rows (and v/spikes)
# so that the LIF state tile ends up as rs[p, c] = local in_feature 8p + c --
# then flattening spk [128, 8] to the spikes row [1, 1024] is the identity
# (p, c) iteration, a plain contiguous SBUF->SBUF DMA with no transpose.
X_ROWS_PER_PART = 4
X_TILES = SHARD // (128 * X_ROWS_PER_PART)  # 2 x-tiles, one per HWDGE ring
T_COLS = SHARD // 128                   # 8 state columns

# host permutation: x_perm[j] = x[PERM[j]]; load AP puts perm row
# 512t + 128a + p on partition p, state column c = 4t + a, and we need
# state (p, c) == original in_feature 8p + c.
_J = np.arange(SHARD)
PERM = 8 * (_J % 128) + _J // 128


def _build_bass(
    reps: int = 1,
    rows_per_part: int = ROWS_PER_PART,
    wbufs: int = 8,
    fake_spikes: bool = False,
    loop_reps: int = 0,
    loop_unroll: int = 8,
    graded: bool = False,
    spec: int = 0,
) -> bass.Bass:
    """reps>1 repeats the phase-2 weight stream (for HW timing via deltas);
    output is identical since every pass writes the same values.

    loop_reps>0 instead wraps `loop_unroll` unrolled passes in a tc.For_i
    hardware loop executed loop_reps times (loop_reps*loop_unroll passes
    total at constant instruction count) -- the robust HW timing variant:
    per-NEFF device time can be pushed to ~100ms so axon dispatch jitter
    (+-tens of ms) divides out, while the NEFF stays small enough that
    instruction fetch never leaves IRAM."""
    if graded:
        # small head/tail tiles: first store starts ~4x sooner, last-store
        # drain ~4x shorter; middle tiles carry the stream
        n_mid = (OUT_FEATURES // 128 - 8) // rows_per_part
        pattern = [1, 1, 2] + [rows_per_part] * n_mid + [2, 1, 1]
    else:
        pattern = [rows_per_part] * (OUT_FEATURES // (128 * rows_per_part))
    assert sum(pattern) * 128 == OUT_FEATURES
    segments = []          # (row_start, rows_per_part)
    row0 = 0
    for rpp in pattern:
        segments.append((row0, rpp))
        row0 += 128 * rpp

    nc = bacc.Bacc(
        "TRN2",
        target_bir_lowering=False,
        debug=False,
        num_devices=N_CORES,
    )

    x = nc.dram_tensor("x", [SHARD, K], BF16, kind="ExternalInput")
    w = nc.dram_tensor("w", [OUT_FEATURES, SHARD], BF16, kind="ExternalInput")
    v = nc.dram_tensor("v", [128, T_COLS], F32, kind="ExternalInput")
    s = nc.dram_tensor("s", [128, T_COLS], F32, kind="ExternalInput")
    o = nc.dram_tensor("o", [OUT_FEATURES, SHARD], BF16, kind="ExternalOutput")

    with TileContext(nc) as tc:
        with (
            tc.tile_pool(name="state", bufs=1) as state,
            tc.tile_pool(name="xp", bufs=2) as xp,
            tc.tile_pool(name="wp", bufs=wbufs) as wp,
            tc.tile_pool(name="ws", bufs=max(spec, 1)) as ws_pool,
        ):
            # ---- Phase 1: LIF state -> broadcast spike row ----
            if spec:
                # speculative all-ones spike row, ready at t~0: the first
                # `spec` weight tiles multiply by this (a no-op product) so
                # stores start immediately; conditional corrections later
                # overwrite them in store-ring FIFO order iff any neuron
                # did not fire.
                bb0 = state.tile([128, SHARD], BF16)
                nc.vector.memset(bb0[:], 1.0)

            if fake_spikes:
                # timing-only variant: skip the LIF computation to isolate
                # phase-1's contribution to the sim/HW overhead
                bb = state.tile([128, SHARD], BF16)
                nc.vector.memset(bb[:], 1.0)

            rs = state.tile([128, T_COLS], F32)
            for t in range(X_TILES) if not fake_spikes else []:
                xt = xp.tile([128, X_ROWS_PER_PART, K], BF16)
                # rows a*128 + p for a in range(X_ROWS_PER_PART)
                src = x[t * 128 * X_ROWS_PER_PART:(t + 1) * 128 * X_ROWS_PER_PART, :]
                src = src.rearrange("(a p) c -> p a c", p=128)
                # alternate HWDGE rings so the x tiles arrive in parallel
                dma_eng = nc.sync if t % 2 == 0 else nc.scalar
                dma_eng.dma_start(out=xt[:], in_=src)
                nc.vector.reduce_sum(
                    out=rs[:, t * X_ROWS_PER_PART:(t + 1) * X_ROWS_PER_PART],
                    in_=xt[:],
                    axis=mybir.AxisListType.X,
                )

            if not fake_spikes:
                vt = state.tile([128, T_COLS], F32)
                st = state.tile([128, T_COLS], F32)
                nc.gpsimd.dma_start(out=vt[:], in_=v[:])
                nc.gpsimd.dma_start(out=st[:], in_=s[:])

                # Everything that depends only on v/s runs while x streams:
                # fired = (v*DECAY + rs*10 >= THR)  <=>  rs >= (THR - v*DECAY)/10
                thr = state.tile([128, T_COLS], F32)
                nc.vector.tensor_scalar(
                    out=thr[:],
                    in0=vt[:],
                    scalar1=-DECAY / 10.0,
                    scalar2=THRESHOLD / 10.0,
                    op0=mybir.AluOpType.mult,
                    op1=mybir.AluOpType.add,
                )
                ones = state.tile([128, T_COLS], F32)
                nc.vector.memset(ones[:], 1.0)
                spk = state.tile([128, T_COLS], F32)
                nc.vector.tensor_copy(out=spk[:], in_=st[:])

                # critical path after the last x reduce: compare + select
                mask = state.tile([128, T_COLS], mybir.dt.uint32)
                nc.vector.tensor_tensor(
                    out=mask[:], in0=rs[:], in1=thr[:], op=mybir.AluOpType.is_ge
                )
                nc.vector.copy_predicated(spk[:], mask[:], ones[:])

                # flatten spk [128, T_COLS] -> bf16 row [1, SHARD], casting in
                # the SWDGE DMA.  Thanks to the host permutation this is the
                # identity iteration order (128 x 32B descriptors).  SWDGE so
                # the HWDGE rings never stall on this compute-dependent DMA.
                rowh = state.tile([1, SHARD], BF16)
                nc.gpsimd.dma_start(out=rowh[:1, :], in_=spk[:])
                bb = state.tile([128, SHARD], BF16)
                nc.gpsimd.partition_broadcast(bb[:], rowh[:1, :])

            bb_row = bb[:, :].rearrange("p (z c) -> p z c", z=1)
            bb_bcast = {
                rpp: bb_row.broadcast_to([128, rpp, SHARD])
                for rpp in set(pattern)
            }
            if spec:
                assert not fake_spikes
                bb0_row = bb0[:, :].rearrange("p (z c) -> p z c", z=1)
                bb0_bcast = {
                    rpp: bb0_row.broadcast_to([128, rpp, SHARD])
                    for rpp in set(pattern)
                }
                # correction trigger: did any neuron not fire?
                # min(spikes_new) as int32 [1,1] (value_load needs an int
                # source): cast, free-axis min, flatten, min again
                spki = state.tile([128, T_COLS], mybir.dt.int32)
                nc.vector.tensor_copy(out=spki[:], in_=spk[:])
                minp = state.tile([128, 1], mybir.dt.int32)
                nc.vector.tensor_reduce(
                    out=minp[:], in_=spki[:],
                    axis=mybir.AxisListType.X, op=mybir.AluOpType.min,
                )
                mrow = state.tile([1, 128], mybir.dt.int32)
                nc.gpsimd.dma_start(out=mrow[:1, :], in_=minp[:])
                mflat = state.tile([1, 1], mybir.dt.int32)
                nc.vector.tensor_reduce(
                    out=mflat[:1, :], in_=mrow[:1, :],
                    axis=mybir.AxisListType.X, op=mybir.AluOpType.min,
                )

            # ---- Phase 2: out = weight * spikes (column-broadcast) ----
            def seg_load(row0, rpp, pool, tag):
                wt = pool.tile([128, rpp * SHARD], BF16, tag=tag)
                src = w[row0:row0 + 128 * rpp, :]
                src = src.rearrange("(p a) c -> p (a c)", a=rpp)
                nc.sync.dma_start(out=wt[:], in_=src)
                return wt

            def seg_mul(wt, rpp, bcast):
                nc.vector.tensor_mul(
                    out=wt[:].rearrange("p (a c) -> p a c", a=rpp),
                    in0=wt[:].rearrange("p (a c) -> p a c", a=rpp),
                    in1=bcast[rpp],
                )

            def seg_store(wt, row0, rpp, cond=None):
                dst = o[row0:row0 + 128 * rpp, :]
                dst = dst.rearrange("(p a) c -> p (a c)", a=rpp)
                if cond is None:
                    nc.scalar.dma_start(out=dst, in_=wt[:])
                else:
                    nc.scalar.dma_start(out=dst, in_=wt[:], cond=cond,
                                        cond_hint=False)

            def emit_pass(sgs, spec_k=0):
                # With spec_k > 0, the first spec_k tiles multiply by the
                # all-ones row (exact no-op on the bf16 weights) and store
                # immediately; their tiles stay resident in the ws pool.
                # Once the true spike row exists, each is re-multiplied
                # in place and conditionally re-stored on the same store
                # ring (FIFO => overwrites the speculative bytes) iff some
                # neuron did not fire.  Corrections are spread through the
                # stream so they never stall the store ring or the DVE.
                spec_tiles = []
                fixed = 0
                cond = None

                def emit_fix():
                    swt, srow0, srpp = spec_tiles[fixed]
                    seg_mul(swt, srpp, bb_bcast)
                    seg_store(swt, srow0, srpp, cond=cond)

                for i, (row0, rpp) in enumerate(sgs):
                    if i < spec_k:
                        wt = seg_load(row0, rpp, ws_pool, "ws")
                        seg_mul(wt, rpp, bb0_bcast)
                        seg_store(wt, row0, rpp)
                        spec_tiles.append((wt, row0, rpp))
                        continue
                    wt = seg_load(row0, rpp, wp, "wt")
                    seg_mul(wt, rpp, bb_bcast)
                    seg_store(wt, row0, rpp)
                    if spec_k:
                        if i == spec_k + 1:
                            val = nc.scalar.value_load(
                                mflat[:1, :1], min_val=0, max_val=1
                            )
                            one = nc.scalar.compute_val(1)
                            cond = val != one
                            assert not isinstance(cond, bool), (
                                "ScalarValue __ne__ fell back to identity"
                            )
                        if (
                            i >= spec_k + 2
                            and (i - spec_k) % 2 == 0
                            and fixed < len(spec_tiles)
                        ):
                            emit_fix()
                            fixed += 1
                while spec_k and fixed < len(spec_tiles):
                    emit_fix()
                    fixed += 1

            if loop_reps > 0:
                with tc.For_i(0, loop_reps, 1):
                    emit_pass([sg for _ in range(loop_unroll) for sg in segments])
            else:
                for rep in range(reps):
                    emit_pass(list(segments), spec_k=spec if rep == 0 else 0)

    nc.compile()
    return nc


_NC_CACHE = {}


def _get_bass(reps: int = 1, **kwargs) -> bass.Bass:
    key = (reps, tuple(sorted(kwargs.items())))
    if key not in _NC_CACHE:
        _NC_CACHE[key] = _build_bass(reps, **kwargs)
    return _NC_CACHE[key]


def _shard_inputs(x, weight, v, spikes):
    x16 = x.astype(NP_BF16)
    w16 = weight.astype(NP_BF16)
    in_maps = []
    for j in range(N_CORES):
        sl = slice(j * SHARD, (j + 1) * SHARD)
        in_maps.append({
            "x": np.ascontiguousarray(x16[sl, :][PERM]),
            "w": np.ascontiguousarray(w16[:, sl]),
            "v": np.ascontiguousarray(v[sl].reshape(128, T_COLS)),
            "s": np.ascontiguousarray(spikes[sl].reshape(128, T_COLS)),
        })
    return in_maps


def run(x, weight, v, spikes, trace=False, **run_kwargs):
    """Run the 8-core kernel; returns (full_output, BassKernelResults)."""
    x = np.asarray(x, dtype=np.float32)
    weight = np.asarray(weight, dtype=np.float32)
    v = np.asarray(v, dtype=np.float32)
    spikes = np.asarray(spikes, dtype=np.float32)
    assert x.shape == (IN_FEATURES, K)
    assert weight.shape == (OUT_FEATURES, IN_FEATURES)

    nc = _get_bass()
    in_maps = _shard_inputs(x, weight, v, spikes)
    res = run_bass_kernel_spmd(
        nc, in_maps, core_ids=list(range(N_CORES)), trace=trace, **run_kwargs
    )
    out = np.empty((OUT_FEATURES, IN_FEATURES), dtype=np.float32)
    for j in range(N_CORES):
        out[:, j * SHARD:(j + 1) * SHARD] = res.results[j]["o"].astype(np.float32)
    return out, res


def kernel(x, weight, v, spikes, t=None, **_ignored):
    out, _ = run(x, weight, v, spikes, trace=False)
    return out


# revision 33
# speedup vs baseline: 1.2675x; 1.0683x over previous
"""LIF neuron kernel for Trainium2 (Bass/Tile), 8-core SPMD, bf16 streaming.

Reference computation (per problem nn_LIF_69707319214329):
    v_new      = v * DECAY + sum(x, axis=1) * 10         # [IN]
    fired      = v_new >= THRESHOLD                      # [IN]
    spikes_new = where(fired, 1.0, spikes)               # [IN]
    out        = spikes_new[None, :] * weight            # [OUT, IN]

Sharding: in_features (columns of weight / rows of x) are split into 8
contiguous blocks of 1024.  Core j receives x rows [1024j, 1024j+1024),
the matching v/spikes slices, and weight[:, block] (made contiguous on the
host).  Each core computes its own spikes slice locally -- no collectives --
and produces out[:, block].

Precision: the correctness gate is rel_err < 2e-2 while bf16 quantization
error is <= 2^-9 ~= 0.2%, so the weight is streamed in bf16 and the output
is produced in bf16 (upcast to fp32 on the host).  spikes_new is exactly
{0.0, 1.0}, so the multiply adds no error beyond the bf16 rounding of
weight.  Per-core HBM traffic: 2MB x + 16MB weight read + 16MB output
write = 34MB (vs 68MB all-fp32), against the ~358 GB/s HBM-per-core limit
(measured ~370-390 GB/s effective with concurrent load+store streams).

Engine plan: weight loads stream on the sync HWDGE ring, output stores on
the scalar HWDGE ring (one x tile on each ring first); small phase-1 DMAs
ride gpsimd SWDGE so neither HWDGE sequencer stalls behind a compute
dependency.  Phase 1 (PE1=True) computes the row sums with a ones-stationary
PE matmul over host-transposed x: the sums land REPLICATED across all 128
PSUM partitions, so the spike row needs no cross-partition flatten or
partition-broadcast on the critical path -- just two DVE tensor-tensor ops
(is_ge against a precomputed per-feature threshold row, then max with the
old spikes row, both broadcast off-path while x streams).
"""

import math

import numpy as np
import ml_dtypes

import concourse.bass as bass
import concourse.bacc as bacc
import concourse.mybir as mybir
from concourse.tile import TileContext
from concourse.bass_utils import run_bass_kernel_spmd

N_CORES = 8
IN_FEATURES = 8192
OUT_FEATURES = 8192
K = 1024
SHARD = IN_FEATURES // N_CORES          # 1024 in_features per core
TAU = 1.0
THRESHOLD = 20.0
DECAY = math.exp(-0.01 / TAU)

F32 = mybir.dt.float32
BF16 = mybir.dt.bfloat16
NP_BF16 = ml_dtypes.bfloat16
W_BYTES = 2                             # weight/output stream dtype size
X_BYTES = 2                             # x stream dtype size

# Main-loop tiling: weight shard [8192, 1024] bf16 seen as ROW_TILES tiles of
# [128, ROWS_PER_PART * 1024]; partition p of tile r holds weight rows
# r*ROWS_PER_TILE + p*ROWS_PER_PART ... + ROWS_PER_PART-1 (contiguous bytes).
# 1MB tiles (rpp=4): same steady-state rate as 2MB in sim, shorter ramp/drain.
ROWS_PER_PART = 4
ROWS_PER_TILE = 128 * ROWS_PER_PART     # 1024
ROW_TILES = OUT_FEATURES // ROWS_PER_TILE

# x shard [1024, 1024] bf16 loaded as X_TILES tiles of
# [128, X_ROWS_PER_PART*1024].  The host pre-permutes x rows (and v/spikes)
# so that the LIF state tile ends up as rs[p, c] = local in_feature 8p + c --
# then flattening spk [128, 8] to the spikes row [1, 1024] is the identity
# (p, c) iteration, a plain contiguous SBUF->SBUF DMA with no transpose.
X_ROWS_PER_PART = 4
X_TILES = SHARD // (128 * X_ROWS_PER_PART)  # 2 x-tiles, one per HWDGE ring
T_COLS = SHARD // 128                   # 8 state columns

# host permutation: x_perm[j] = x[PERM[j]]; load AP puts perm row
# 512t + 128a + p on partition p, state column c = 4t + a, and we need
# state (p, c) == original in_feature 8p + c.
_J = np.arange(SHARD)
PERM = 8 * (_J % 128) + _J // 128

# PE-based phase 1 (row sums via ones-stationary matmul, replicated across
# partitions in PSUM -- no flatten/broadcast on the critical path)
PE1 = True


def _build_bass(
    reps: int = 1,
    rows_per_part: int = ROWS_PER_PART,
    wbufs: int = 8,
    fake_spikes: bool = False,
    loop_reps: int = 0,
    loop_unroll: int = 8,
    graded: bool = False,
    spec: int = 0,
    pe1: bool | None = None,
) -> bass.Bass:
    """reps>1 repeats the phase-2 weight stream (for HW timing via deltas);
    output is identical since every pass writes the same values.

    loop_reps>0 instead wraps `loop_unroll` unrolled passes in a tc.For_i
    hardware loop executed loop_reps times (loop_reps*loop_unroll passes
    total at constant instruction count) -- the robust HW timing variant:
    per-NEFF device time can be pushed to ~100ms so axon dispatch jitter
    (+-tens of ms) divides out, while the NEFF stays small enough that
    instruction fetch never leaves IRAM."""
    if pe1 is None:
        pe1 = PE1
    if graded:
        # small head/tail tiles: first store starts ~4x sooner, last-store
        # drain ~4x shorter; middle tiles carry the stream
        n_mid = (OUT_FEATURES // 128 - 8) // rows_per_part
        pattern = [1, 1, 2] + [rows_per_part] * n_mid + [2, 1, 1]
    else:
        pattern = [rows_per_part] * (OUT_FEATURES // (128 * rows_per_part))
    assert sum(pattern) * 128 == OUT_FEATURES
    segments = []          # (row_start, rows_per_part)
    row0 = 0
    for rpp in pattern:
        segments.append((row0, rpp))
        row0 += 128 * rpp

    nc = bacc.Bacc(
        "TRN2",
        target_bir_lowering=False,
        debug=False,
        num_devices=N_CORES,
    )

    if pe1:
        # x arrives transposed [K, SHARD] (features along the free axis, in
        # natural order -- no PERM); v/s as [1, SHARD] rows
        x = nc.dram_tensor("x", [K, SHARD], BF16, kind="ExternalInput")
        v = nc.dram_tensor("v", [1, SHARD], F32, kind="ExternalInput")
        s = nc.dram_tensor("s", [1, SHARD], F32, kind="ExternalInput")
    else:
        x = nc.dram_tensor("x", [SHARD, K], BF16, kind="ExternalInput")
        v = nc.dram_tensor("v", [128, T_COLS], F32, kind="ExternalInput")
        s = nc.dram_tensor("s", [128, T_COLS], F32, kind="ExternalInput")
    w = nc.dram_tensor("w", [OUT_FEATURES, SHARD], BF16, kind="ExternalInput")
    o = nc.dram_tensor("o", [OUT_FEATURES, SHARD], BF16, kind="ExternalOutput")

    with TileContext(nc) as tc:
        with (
            tc.tile_pool(name="state", bufs=1) as state,
            tc.tile_pool(name="xp", bufs=2) as xp,
            tc.tile_pool(name="wp", bufs=wbufs) as wp,
            tc.tile_pool(name="ws", bufs=max(spec, 1)) as ws_pool,
            tc.tile_pool(name="ps", bufs=1, space="PSUM") as psum_pool,
        ):
            # ---- Phase 1: LIF state -> broadcast spike row ----
            if spec:
                # speculative all-ones spike row, ready at t~0: the first
                # `spec` weight tiles multiply by this (a no-op product) so
                # stores start immediately; conditional corrections later
                # overwrite them in store-ring FIFO order iff any neuron
                # did not fire.
                bb0 = state.tile([128, SHARD], BF16)
                nc.vector.memset(bb0[:], 1.0)

            if fake_spikes:
                # timing-only variant: skip the LIF computation to isolate
                # phase-1's contribution to the sim/HW overhead
                bb = state.tile([128, SHARD], BF16)
                nc.vector.memset(bb[:], 1.0)

            if pe1 and not fake_spikes:
                assert not spec
                # --- PE-based phase 1: row sums land REPLICATED on all 128
                # partitions, so no flatten / partition-broadcast sits on the
                # critical path.  out[m, f] = sum_k ones[k, m] * xT[k, f].
                ones_t = state.tile([128, 128], BF16)
                nc.vector.memset(ones_t[:], 1.0)
                vt = state.tile([1, SHARD], F32)
                st = state.tile([1, SHARD], F32)
                nc.gpsimd.dma_start(out=vt[:1, :], in_=v[:])
                nc.gpsimd.dma_start(out=st[:1, :], in_=s[:])
                # fired = (v*DECAY + sums*10 >= THR) <=> sums >= thr;
                # thr/spikes rows broadcast off the critical path
                thr1 = state.tile([1, SHARD], F32)
                nc.vector.tensor_scalar(
                    out=thr1[:1, :],
                    in0=vt[:1, :],
                    scalar1=-DECAY / 10.0,
                    scalar2=THRESHOLD / 10.0,
                    op0=mybir.AluOpType.mult,
                    op1=mybir.AluOpType.add,
                )
                thr_b = state.tile([128, SHARD], F32)
                nc.gpsimd.partition_broadcast(thr_b[:], thr1[:1, :])
                s16 = state.tile([1, SHARD], BF16)
                nc.vector.tensor_copy(out=s16[:1, :], in_=st[:1, :])
                st_b = state.tile([128, SHARD], BF16)
                nc.gpsimd.partition_broadcast(st_b[:], s16[:1, :])

                XKC = 2                      # k-chunks of 128 per x tile
                HALF = SHARD // 2            # psum bank width in fp32
                psum = psum_pool.tile([128, SHARD], F32)
                n_xt = K // (128 * XKC)
                for t in range(n_xt):
                    xt = xp.tile([128, XKC, SHARD], BF16)
                    src = x[t * 128 * XKC:(t + 1) * 128 * XKC, :]
                    src = src.rearrange("(a p) c -> p a c", p=128)
                    dma_eng = nc.sync if t % 2 == 0 else nc.scalar
                    dma_eng.dma_start(out=xt[:], in_=src)
                    for a in range(XKC):
                        ki = t * XKC + a
                        for h in range(2):
                            nc.tensor.matmul(
                                psum[:, h * HALF:(h + 1) * HALF],
                                ones_t[:],
                                xt[:, a, h * HALF:(h + 1) * HALF],
                                start=(ki == 0),
                                stop=(ki == K // 128 - 1),
                            )

                fired16 = state.tile([128, SHARD], BF16)
                nc.vector.tensor_tensor(
                    out=fired16[:], in0=psum[:], in1=thr_b[:],
                    op=mybir.AluOpType.is_ge,
                )
                bb = state.tile([128, SHARD], BF16)
                nc.vector.tensor_tensor(
                    out=bb[:], in0=fired16[:], in1=st_b[:],
                    op=mybir.AluOpType.max,
                )

            rs = state.tile([128, T_COLS], F32)
            for t in range(X_TILES) if not (fake_spikes or pe1) else []:
                xt = xp.tile([128, X_ROWS_PER_PART, K], BF16)
                # rows a*128 + p for a in range(X_ROWS_PER_PART)
                src = x[t * 128 * X_ROWS_PER_PART:(t + 1) * 128 * X_ROWS_PER_PART, :]
                src = src.rearrange("(a p) c -> p a c", p=128)
                # alternate HWDGE rings so the x tiles arrive in parallel
                dma_eng = nc.sync if t % 2 == 0 else nc.scalar
                dma_eng.dma_start(out=xt[:], in_=src)
                nc.vector.reduce_sum(
                    out=rs[:, t * X_ROWS_PER_PART:(t + 1) * X_ROWS_PER_PART],
                    in_=xt[:],
                    axis=mybir.AxisListType.X,
                )

            if not fake_spikes and not pe1:
                vt = state.tile([128, T_COLS], F32)
                st = state.tile([128, T_COLS], F32)
                nc.gpsimd.dma_start(out=vt[:], in_=v[:])
                nc.gpsimd.dma_start(out=st[:], in_=s[:])

                # Everything that depends only on v/s runs while x streams:
                # fired = (v*DECAY + rs*10 >= THR)  <=>  rs >= (THR - v*DECAY)/10
                thr = state.tile([128, T_COLS], F32)
                nc.vector.tensor_scalar(
                    out=thr[:],
                    in0=vt[:],
                    scalar1=-DECAY / 10.0,
                    scalar2=THRESHOLD / 10.0,
                    op0=mybir.AluOpType.mult,
                    op1=mybir.AluOpType.add,
                )
                ones = state.tile([128, T_COLS], F32)
                nc.vector.memset(ones[:], 1.0)
                spk = state.tile([128, T_COLS], F32)
                nc.vector.tensor_copy(out=spk[:], in_=st[:])

                # critical path after the last x reduce: compare + select
                mask = state.tile([128, T_COLS], mybir.dt.uint32)
                nc.vector.tensor_tensor(
                    out=mask[:], in0=rs[:], in1=thr[:], op=mybir.AluOpType.is_ge
                )
                nc.vector.copy_predicated(spk[:], mask[:], ones[:])

                # flatten spk [128, T_COLS] -> bf16 row [1, SHARD], casting in
                # the SWDGE DMA.  Thanks to the host permutation this is the
                # identity iteration order (128 x 32B descriptors).  SWDGE so
                # the HWDGE rings never stall on this compute-dependent DMA.
                rowh = state.tile([1, SHARD], BF16)
                nc.gpsimd.dma_start(out=rowh[:1, :], in_=spk[:])
                bb = state.tile([128, SHARD], BF16)
                nc.gpsimd.partition_broadcast(bb[:], rowh[:1, :])

            bb_row = bb[:, :].rearrange("p (z c) -> p z c", z=1)
            bb_bcast = {
                rpp: bb_row.broadcast_to([128, rpp, SHARD])
                for rpp in set(pattern)
            }
            if spec:
                assert not fake_spikes
                bb0_row = bb0[:, :].rearrange("p (z c) -> p z c", z=1)
                bb0_bcast = {
                    rpp: bb0_row.broadcast_to([128, rpp, SHARD])
                    for rpp in set(pattern)
                }
                # correction trigger: did any neuron not fire?
                # min(spikes_new) as int32 [1,1] (value_load needs an int
                # source): cast, free-axis min, flatten, min again
                spki = state.tile([128, T_COLS], mybir.dt.int32)
                nc.vector.tensor_copy(out=spki[:], in_=spk[:])
                minp = state.tile([128, 1], mybir.dt.int32)
                nc.vector.tensor_reduce(
                    out=minp[:], in_=spki[:],
                    axis=mybir.AxisListType.X, op=mybir.AluOpType.min,
                )
                mrow = state.tile([1, 128], mybir.dt.int32)
                nc.gpsimd.dma_start(out=mrow[:1, :], in_=minp[:])
                mflat = state.tile([1, 1], mybir.dt.int32)
                nc.vector.tensor_reduce(
                    out=mflat[:1, :], in_=mrow[:1, :],
                    axis=mybir.AxisListType.X, op=mybir.AluOpType.min,
                )

            # ---- Phase 2: out = weight * spikes (column-broadcast) ----
            def seg_load(row0, rpp, pool, tag):
                wt = pool.tile([128, rpp * SHARD], BF16, tag=tag)
                src = w[row0:row0 + 128 * rpp, :]
                src = src.rearrange("(p a) c -> p (a c)", a=rpp)
                nc.sync.dma_start(out=wt[:], in_=src)
                return wt

            def seg_mul(wt, rpp, bcast):
                nc.vector.tensor_mul(
                    out=wt[:].rearrange("p (a c) -> p a c", a=rpp),
                    in0=wt[:].rearrange("p (a c) -> p a c", a=rpp),
                    in1=bcast[rpp],
                )

            def seg_store(wt, row0, rpp, cond=None):
                dst = o[row0:row0 + 128 * rpp, :]
                dst = dst.rearrange("(p a) c -> p (a c)", a=rpp)
                if cond is None:
                    nc.scalar.dma_start(out=dst, in_=wt[:])
                else:
                    nc.scalar.dma_start(out=dst, in_=wt[:], cond=cond,
                                        cond_hint=False)

            def emit_pass(sgs, spec_k=0):
                # With spec_k > 0, the first spec_k tiles multiply by the
                # all-ones row (exact no-op on the bf16 weights) and store
                # immediately; their tiles stay resident in the ws pool.
                # Once the true spike row exists, each is re-multiplied
                # in place and conditionally re-stored on the same store
                # ring (FIFO => overwrites the speculative bytes) iff some
                # neuron did not fire.  Corrections are spread through the
                # stream so they never stall the store ring or the DVE.
                spec_tiles = []
                fixed = 0
                cond = None

                def emit_fix():
                    swt, srow0, srpp = spec_tiles[fixed]
                    seg_mul(swt, srpp, bb_bcast)
                    seg_store(swt, srow0, srpp, cond=cond)

                for i, (row0, rpp) in enumerate(sgs):
                    if i < spec_k:
                        wt = seg_load(row0, rpp, ws_pool, "ws")
                        seg_mul(wt, rpp, bb0_bcast)
                        seg_store(wt, row0, rpp)
                        spec_tiles.append((wt, row0, rpp))
                        continue
                    wt = seg_load(row0, rpp, wp, "wt")
                    seg_mul(wt, rpp, bb_bcast)
                    seg_store(wt, row0, rpp)
                    if spec_k:
                        if i == spec_k + 1:
                            val = nc.scalar.value_load(
                                mflat[:1, :1], min_val=0, max_val=1
                            )
                            one = nc.scalar.compute_val(1)
                            cond = val != one
                            assert not isinstance(cond, bool), (
                                "ScalarValue __ne__ fell back to identity"
                            )
                        if (
                            i >= spec_k + 2
                            and (i - spec_k) % 2 == 0
                            and fixed < len(spec_tiles)
                        ):
                            emit_fix()
                            fixed += 1
                while spec_k and fixed < len(spec_tiles):
                    emit_fix()
                    fixed += 1

            if loop_reps > 0:
                with tc.For_i(0, loop_reps, 1):
                    emit_pass([sg for _ in range(loop_unroll) for sg in segments])
            else:
                for rep in range(reps):
                    emit_pass(list(segments), spec_k=spec if rep == 0 else 0)

    nc.compile()
    return nc


_NC_CACHE = {}


def _get_bass(reps: int = 1, **kwargs) -> bass.Bass:
    key = (reps, tuple(sorted(kwargs.items())))
    if key not in _NC_CACHE:
        _NC_CACHE[key] = _build_bass(reps, **kwargs)
    return _NC_CACHE[key]


def _shard_inputs(x, weight, v, spikes, pe1=None):
    if pe1 is None:
        pe1 = PE1
    x16 = x.astype(NP_BF16)
    w16 = weight.astype(NP_BF16)
    in_maps = []
    for j in range(N_CORES):
        sl = slice(j * SHARD, (j + 1) * SHARD)
        if pe1:
            in_maps.append({
                "x": np.ascontiguousarray(x16[sl, :].T),
                "w": np.ascontiguousarray(w16[:, sl]),
                "v": np.ascontiguousarray(v[sl].reshape(1, SHARD)),
                "s": np.ascontiguousarray(spikes[sl].reshape(1, SHARD)),
            })
        else:
            in_maps.append({
                "x": np.ascontiguousarray(x16[sl, :][PERM]),
                "w": np.ascontiguousarray(w16[:, sl]),
                "v": np.ascontiguousarray(v[sl].reshape(128, T_COLS)),
                "s": np.ascontiguousarray(spikes[sl].reshape(128, T_COLS)),
            })
    return in_maps


def run(x, weight, v, spikes, trace=False, **run_kwargs):
    """Run the 8-core kernel; returns (full_output, BassKernelResults)."""
    x = np.asarray(x, dtype=np.float32)
    weight = np.asarray(weight, dtype=np.float32)
    v = np.asarray(v, dtype=np.float32)
    spikes = np.asarray(spikes, dtype=np.float32)
    assert x.shape == (IN_FEATURES, K)
    assert weight.shape == (OUT_FEATURES, IN_FEATURES)

    nc = _get_bass()
    in_maps = _shard_inputs(x, weight, v, spikes)
    res = run_bass_kernel_spmd(
        nc, in_maps, core_ids=list(range(N_CORES)), trace=trace, **run_kwargs
    )
    out = np.empty((OUT_FEATURES, IN_FEATURES), dtype=np.float32)
    for j in range(N_CORES):
        out[:, j * SHARD:(j + 1) * SHARD] = res.results[j]["o"].astype(np.float32)
    return out, res


def kernel(x, weight, v, spikes, t=None, **_ignored):
    out, _ = run(x, weight, v, spikes, trace=False)
    return out


# revision 36
# speedup vs baseline: 1.2738x; 1.0049x over previous
"""LIF neuron kernel for Trainium2 (Bass/Tile), 8-core SPMD, bf16 streaming.

Reference computation (per problem nn_LIF_69707319214329):
    v_new      = v * DECAY + sum(x, axis=1) * 10         # [IN]
    fired      = v_new >= THRESHOLD                      # [IN]
    spikes_new = where(fired, 1.0, spikes)               # [IN]
    out        = spikes_new[None, :] * weight            # [OUT, IN]

Sharding: in_features (columns of weight / rows of x) are split into 8
contiguous blocks of 1024.  Core j receives x rows [1024j, 1024j+1024),
the matching v/spikes slices, and weight[:, block] (made contiguous on the
host).  Each core computes its own spikes slice locally -- no collectives --
and produces out[:, block].

Precision: the correctness gate is rel_err < 2e-2 while bf16 quantization
error is <= 2^-9 ~= 0.2%, so the weight is streamed in bf16 and the output
is produced in bf16 (upcast to fp32 on the host).  spikes_new is exactly
{0.0, 1.0}, so the multiply adds no error beyond the bf16 rounding of
weight.  Per-core HBM traffic: 2MB x + 16MB weight read + 16MB output
write = 34MB (vs 68MB all-fp32), against the ~358 GB/s HBM-per-core limit
(measured ~370-390 GB/s effective with concurrent load+store streams).

Engine plan: weight loads stream on the sync HWDGE ring, output stores on
the scalar HWDGE ring (one x tile on each ring first); small phase-1 DMAs
ride gpsimd SWDGE so neither HWDGE sequencer stalls behind a compute
dependency.  Phase 1 (PE1=True) computes the row sums with a ones-stationary
PE matmul over host-transposed x: the sums land REPLICATED across all 128
PSUM partitions, so the spike row needs no cross-partition flatten or
partition-broadcast on the critical path -- just two DVE tensor-tensor ops
(is_ge against a precomputed per-feature threshold row, then max with the
old spikes row, both broadcast off-path while x streams).
"""

import math

import numpy as np
import ml_dtypes

import concourse.bass as bass
import concourse.bacc as bacc
import concourse.mybir as mybir
from concourse.tile import TileContext
from concourse.bass_utils import run_bass_kernel_spmd

N_CORES = 8
IN_FEATURES = 8192
OUT_FEATURES = 8192
K = 1024
SHARD = IN_FEATURES // N_CORES          # 1024 in_features per core
TAU = 1.0
THRESHOLD = 20.0
DECAY = math.exp(-0.01 / TAU)

F32 = mybir.dt.float32
BF16 = mybir.dt.bfloat16
NP_BF16 = ml_dtypes.bfloat16
W_BYTES = 2                             # weight/output stream dtype size
X_BYTES = 2                             # x stream dtype size

# Main-loop tiling: weight shard [8192, 1024] bf16 seen as ROW_TILES tiles of
# [128, ROWS_PER_PART * 1024]; partition p of tile r holds weight rows
# r*ROWS_PER_TILE + p*ROWS_PER_PART ... + ROWS_PER_PART-1 (contiguous bytes).
# 1MB tiles (rpp=4): same steady-state rate as 2MB in sim, shorter ramp/drain.
ROWS_PER_PART = 4
ROWS_PER_TILE = 128 * ROWS_PER_PART     # 1024
ROW_TILES = OUT_FEATURES // ROWS_PER_TILE

# x shard [1024, 1024] bf16 loaded as X_TILES tiles of
# [128, X_ROWS_PER_PART*1024].  The host pre-permutes x rows (and v/spikes)
# so that the LIF state tile ends up as rs[p, c] = local in_feature 8p + c --
# then flattening spk [128, 8] to the spikes row [1, 1024] is the identity
# (p, c) iteration, a plain contiguous SBUF->SBUF DMA with no transpose.
X_ROWS_PER_PART = 4
X_TILES = SHARD // (128 * X_ROWS_PER_PART)  # 2 x-tiles, one per HWDGE ring
T_COLS = SHARD // 128                   # 8 state columns

# host permutation: x_perm[j] = x[PERM[j]]; load AP puts perm row
# 512t + 128a + p on partition p, state column c = 4t + a, and we need
# state (p, c) == original in_feature 8p + c.
_J = np.arange(SHARD)
PERM = 8 * (_J % 128) + _J // 128

# PE-based phase 1 (row sums via ones-stationary matmul, replicated across
# partitions in PSUM -- no flatten/broadcast on the critical path)
PE1 = True


def _build_bass(
    reps: int = 1,
    rows_per_part: int = ROWS_PER_PART,
    wbufs: int = 8,
    fake_spikes: bool = False,
    loop_reps: int = 0,
    loop_unroll: int = 8,
    graded: bool = False,
    spec: int = 0,
    pe1: bool | None = None,
    xkc: int = 4,
) -> bass.Bass:
    """reps>1 repeats the phase-2 weight stream (for HW timing via deltas);
    output is identical since every pass writes the same values.

    loop_reps>0 instead wraps `loop_unroll` unrolled passes in a tc.For_i
    hardware loop executed loop_reps times (loop_reps*loop_unroll passes
    total at constant instruction count) -- the robust HW timing variant:
    per-NEFF device time can be pushed to ~100ms so axon dispatch jitter
    (+-tens of ms) divides out, while the NEFF stays small enough that
    instruction fetch never leaves IRAM."""
    if pe1 is None:
        pe1 = PE1
    if graded:
        # small head/tail tiles: first store starts ~4x sooner, last-store
        # drain ~4x shorter; middle tiles carry the stream
        n_mid = (OUT_FEATURES // 128 - 8) // rows_per_part
        pattern = [1, 1, 2] + [rows_per_part] * n_mid + [2, 1, 1]
    else:
        pattern = [rows_per_part] * (OUT_FEATURES // (128 * rows_per_part))
    assert sum(pattern) * 128 == OUT_FEATURES
    segments = []          # (row_start, rows_per_part)
    row0 = 0
    for rpp in pattern:
        segments.append((row0, rpp))
        row0 += 128 * rpp

    nc = bacc.Bacc(
        "TRN2",
        target_bir_lowering=False,
        debug=False,
        num_devices=N_CORES,
    )

    if pe1:
        # x arrives transposed [K, SHARD] (features along the free axis, in
        # natural order -- no PERM); v/s as [1, SHARD] rows
        x = nc.dram_tensor("x", [K, SHARD], BF16, kind="ExternalInput")
        v = nc.dram_tensor("v", [1, SHARD], F32, kind="ExternalInput")
        s = nc.dram_tensor("s", [1, SHARD], F32, kind="ExternalInput")
    else:
        x = nc.dram_tensor("x", [SHARD, K], BF16, kind="ExternalInput")
        v = nc.dram_tensor("v", [128, T_COLS], F32, kind="ExternalInput")
        s = nc.dram_tensor("s", [128, T_COLS], F32, kind="ExternalInput")
    w = nc.dram_tensor("w", [OUT_FEATURES, SHARD], BF16, kind="ExternalInput")
    o = nc.dram_tensor("o", [OUT_FEATURES, SHARD], BF16, kind="ExternalOutput")

    with TileContext(nc) as tc:
        with (
            tc.tile_pool(name="state", bufs=1) as state,
            tc.tile_pool(name="xp", bufs=2) as xp,
            tc.tile_pool(name="wp", bufs=wbufs) as wp,
            tc.tile_pool(name="ws", bufs=max(spec, 1)) as ws_pool,
            tc.tile_pool(name="ps", bufs=1, space="PSUM") as psum_pool,
        ):
            # ---- Phase 1: LIF state -> broadcast spike row ----
            if spec:
                # speculative all-ones spike row, ready at t~0: the first
                # `spec` weight tiles multiply by this (a no-op product) so
                # stores start immediately; conditional corrections later
                # overwrite them in store-ring FIFO order iff any neuron
                # did not fire.
                bb0 = state.tile([128, SHARD], BF16)
                nc.vector.memset(bb0[:], 1.0)

            if fake_spikes:
                # timing-only variant: skip the LIF computation to isolate
                # phase-1's contribution to the sim/HW overhead
                bb = state.tile([128, SHARD], BF16)
                nc.vector.memset(bb[:], 1.0)

            if pe1 and not fake_spikes:
                assert not spec
                # --- PE-based phase 1: row sums land REPLICATED on all 128
                # partitions, so no flatten / partition-broadcast sits on the
                # critical path.  out[m, f] = sum_k ones[k, m] * xT[k, f].
                ones_t = state.tile([128, 128], BF16)
                nc.vector.memset(ones_t[:], 1.0)
                vt = state.tile([1, SHARD], F32)
                st = state.tile([1, SHARD], F32)
                nc.gpsimd.dma_start(out=vt[:1, :], in_=v[:])
                nc.gpsimd.dma_start(out=st[:1, :], in_=s[:])
                # fired = (v*DECAY + sums*10 >= THR) <=> sums >= thr;
                # thr/spikes rows broadcast off the critical path
                thr1 = state.tile([1, SHARD], F32)
                nc.vector.tensor_scalar(
                    out=thr1[:1, :],
                    in0=vt[:1, :],
                    scalar1=-DECAY / 10.0,
                    scalar2=THRESHOLD / 10.0,
                    op0=mybir.AluOpType.mult,
                    op1=mybir.AluOpType.add,
                )
                thr_b = state.tile([128, SHARD], F32)
                nc.gpsimd.partition_broadcast(thr_b[:], thr1[:1, :])
                s16 = state.tile([1, SHARD], BF16)
                nc.vector.tensor_copy(out=s16[:1, :], in_=st[:1, :])
                st_b = state.tile([128, SHARD], BF16)
                nc.gpsimd.partition_broadcast(st_b[:], s16[:1, :])

                XKC = xkc                    # k-chunks of 128 per x tile
                HALF = SHARD // 2            # psum bank width in fp32
                psum = psum_pool.tile([128, SHARD], F32)
                n_xt = K // (128 * XKC)
                for t in range(n_xt):
                    xt = xp.tile([128, XKC, SHARD], BF16)
                    src = x[t * 128 * XKC:(t + 1) * 128 * XKC, :]
                    src = src.rearrange("(a p) c -> p a c", p=128)
                    dma_eng = nc.sync if t % 2 == 0 else nc.scalar
                    dma_eng.dma_start(out=xt[:], in_=src)
                    for a in range(XKC):
                        ki = t * XKC + a
                        for h in range(2):
                            nc.tensor.matmul(
                                psum[:, h * HALF:(h + 1) * HALF],
                                ones_t[:],
                                xt[:, a, h * HALF:(h + 1) * HALF],
                                start=(ki == 0),
                                stop=(ki == K // 128 - 1),
                            )

                fired16 = state.tile([128, SHARD], BF16)
                nc.vector.tensor_tensor(
                    out=fired16[:], in0=psum[:], in1=thr_b[:],
                    op=mybir.AluOpType.is_ge,
                )
                bb = state.tile([128, SHARD], BF16)
                nc.vector.tensor_tensor(
                    out=bb[:], in0=fired16[:], in1=st_b[:],
                    op=mybir.AluOpType.max,
                )

            rs = state.tile([128, T_COLS], F32)
            for t in range(X_TILES) if not (fake_spikes or pe1) else []:
                xt = xp.tile([128, X_ROWS_PER_PART, K], BF16)
                # rows a*128 + p for a in range(X_ROWS_PER_PART)
                src = x[t * 128 * X_ROWS_PER_PART:(t + 1) * 128 * X_ROWS_PER_PART, :]
                src = src.rearrange("(a p) c -> p a c", p=128)
                # alternate HWDGE rings so the x tiles arrive in parallel
                dma_eng = nc.sync if t % 2 == 0 else nc.scalar
                dma_eng.dma_start(out=xt[:], in_=src)
                nc.vector.reduce_sum(
                    out=rs[:, t * X_ROWS_PER_PART:(t + 1) * X_ROWS_PER_PART],
                    in_=xt[:],
                    axis=mybir.AxisListType.X,
                )

            if not fake_spikes and not pe1:
                vt = state.tile([128, T_COLS], F32)
                st = state.tile([128, T_COLS], F32)
                nc.gpsimd.dma_start(out=vt[:], in_=v[:])
                nc.gpsimd.dma_start(out=st[:], in_=s[:])

                # Everything that depends only on v/s runs while x streams:
                # fired = (v*DECAY + rs*10 >= THR)  <=>  rs >= (THR - v*DECAY)/10
                thr = state.tile([128, T_COLS], F32)
                nc.vector.tensor_scalar(
                    out=thr[:],
                    in0=vt[:],
                    scalar1=-DECAY / 10.0,
                    scalar2=THRESHOLD / 10.0,
                    op0=mybir.AluOpType.mult,
                    op1=mybir.AluOpType.add,
                )
                ones = state.tile([128, T_COLS], F32)
                nc.vector.memset(ones[:], 1.0)
                spk = state.tile([128, T_COLS], F32)
                nc.vector.tensor_copy(out=spk[:], in_=st[:])

                # critical path after the last x reduce: compare + select
                mask = state.tile([128, T_COLS], mybir.dt.uint32)
                nc.vector.tensor_tensor(
                    out=mask[:], in0=rs[:], in1=thr[:], op=mybir.AluOpType.is_ge
                )
                nc.vector.copy_predicated(spk[:], mask[:], ones[:])

                # flatten spk [128, T_COLS] -> bf16 row [1, SHARD], casting in
                # the SWDGE DMA.  Thanks to the host permutation this is the
                # identity iteration order (128 x 32B descriptors).  SWDGE so
                # the HWDGE rings never stall on this compute-dependent DMA.
                rowh = state.tile([1, SHARD], BF16)
                nc.gpsimd.dma_start(out=rowh[:1, :], in_=spk[:])
                bb = state.tile([128, SHARD], BF16)
                nc.gpsimd.partition_broadcast(bb[:], rowh[:1, :])

            bb_row = bb[:, :].rearrange("p (z c) -> p z c", z=1)
            bb_bcast = {
                rpp: bb_row.broadcast_to([128, rpp, SHARD])
                for rpp in set(pattern)
            }
            if spec:
                assert not fake_spikes
                bb0_row = bb0[:, :].rearrange("p (z c) -> p z c", z=1)
                bb0_bcast = {
                    rpp: bb0_row.broadcast_to([128, rpp, SHARD])
                    for rpp in set(pattern)
                }
                # correction trigger: did any neuron not fire?
                # min(spikes_new) as int32 [1,1] (value_load needs an int
                # source): cast, free-axis min, flatten, min again
                spki = state.tile([128, T_COLS], mybir.dt.int32)
                nc.vector.tensor_copy(out=spki[:], in_=spk[:])
                minp = state.tile([128, 1], mybir.dt.int32)
                nc.vector.tensor_reduce(
                    out=minp[:], in_=spki[:],
                    axis=mybir.AxisListType.X, op=mybir.AluOpType.min,
                )
                mrow = state.tile([1, 128], mybir.dt.int32)
                nc.gpsimd.dma_start(out=mrow[:1, :], in_=minp[:])
                mflat = state.tile([1, 1], mybir.dt.int32)
                nc.vector.tensor_reduce(
                    out=mflat[:1, :], in_=mrow[:1, :],
                    axis=mybir.AxisListType.X, op=mybir.AluOpType.min,
                )

            # ---- Phase 2: out = weight * spikes (column-broadcast) ----
            def seg_load(row0, rpp, pool, tag):
                wt = pool.tile([128, rpp * SHARD], BF16, tag=tag)
                src = w[row0:row0 + 128 * rpp, :]
                src = src.rearrange("(p a) c -> p (a c)", a=rpp)
                nc.sync.dma_start(out=wt[:], in_=src)
                return wt

            def seg_mul(wt, rpp, bcast):
                nc.vector.tensor_mul(
                    out=wt[:].rearrange("p (a c) -> p a c", a=rpp),
                    in0=wt[:].rearrange("p (a c) -> p a c", a=rpp),
                    in1=bcast[rpp],
                )

            def seg_store(wt, row0, rpp, cond=None):
                dst = o[row0:row0 + 128 * rpp, :]
                dst = dst.rearrange("(p a) c -> p (a c)", a=rpp)
                if cond is None:
                    nc.scalar.dma_start(out=dst, in_=wt[:])
                else:
                    nc.scalar.dma_start(out=dst, in_=wt[:], cond=cond,
                                        cond_hint=False)

            def emit_pass(sgs, spec_k=0):
                # With spec_k > 0, the first spec_k tiles multiply by the
                # all-ones row (exact no-op on the bf16 weights) and store
                # immediately; their tiles stay resident in the ws pool.
                # Once the true spike row exists, each is re-multiplied
                # in place and conditionally re-stored on the same store
                # ring (FIFO => overwrites the speculative bytes) iff some
                # neuron did not fire.  Corrections are spread through the
                # stream so they never stall the store ring or the DVE.
                spec_tiles = []
                fixed = 0
                cond = None

                def emit_fix():
                    swt, srow0, srpp = spec_tiles[fixed]
                    seg_mul(swt, srpp, bb_bcast)
                    seg_store(swt, srow0, srpp, cond=cond)

                for i, (row0, rpp) in enumerate(sgs):
                    if i < spec_k:
                        wt = seg_load(row0, rpp, ws_pool, "ws")
                        seg_mul(wt, rpp, bb0_bcast)
                        seg_store(wt, row0, rpp)
                        spec_tiles.append((wt, row0, rpp))
                        continue
                    wt = seg_load(row0, rpp, wp, "wt")
                    seg_mul(wt, rpp, bb_bcast)
                    seg_store(wt, row0, rpp)
                    if spec_k:
                        if i == spec_k + 1:
                            val = nc.scalar.value_load(
                                mflat[:1, :1], min_val=0, max_val=1
                            )
                            one = nc.scalar.compute_val(1)
                            cond = val != one
                            assert not isinstance(cond, bool), (
                                "ScalarValue __ne__ fell back to identity"
                            )
                        if (
                            i >= spec_k + 2
                            and (i - spec_k) % 2 == 0
                            and fixed < len(spec_tiles)
                        ):
                            emit_fix()
                            fixed += 1
                while spec_k and fixed < len(spec_tiles):
                    emit_fix()
                    fixed += 1

            if loop_reps > 0:
                with tc.For_i(0, loop_reps, 1):
                    emit_pass([sg for _ in range(loop_unroll) for sg in segments])
            else:
                for rep in range(reps):
                    emit_pass(list(segments), spec_k=spec if rep == 0 else 0)

    nc.compile()
    return nc


_NC_CACHE = {}


def _get_bass(reps: int = 1, **kwargs) -> bass.Bass:
    key = (reps, tuple(sorted(kwargs.items())))
    if key not in _NC_CACHE:
        _NC_CACHE[key] = _build_bass(reps, **kwargs)
    return _NC_CACHE[key]


def _shard_inputs(x, weight, v, spikes, pe1=None):
    if pe1 is None:
        pe1 = PE1
    x16 = x.astype(NP_BF16)
    w16 = weight.astype(NP_BF16)
    in_maps = []
    for j in range(N_CORES):
        sl = slice(j * SHARD, (j + 1) * SHARD)
        if pe1:
            in_maps.append({
                "x": np.ascontiguousarray(x16[sl, :].T),
                "w": np.ascontiguousarray(w16[:, sl]),
                "v": np.ascontiguousarray(v[sl].reshape(1, SHARD)),
                "s": np.ascontiguousarray(spikes[sl].reshape(1, SHARD)),
            })
        else:
            in_maps.append({
                "x": np.ascontiguousarray(x16[sl, :][PERM]),
                "w": np.ascontiguousarray(w16[:, sl]),
                "v": np.ascontiguousarray(v[sl].reshape(128, T_COLS)),
                "s": np.ascontiguousarray(spikes[sl].reshape(128, T_COLS)),
            })
    return in_maps


def run(x, weight, v, spikes, trace=False, **run_kwargs):
    """Run the 8-core kernel; returns (full_output, BassKernelResults)."""
    x = np.asarray(x, dtype=np.float32)
    weight = np.asarray(weight, dtype=np.float32)
    v = np.asarray(v, dtype=np.float32)
    spikes = np.asarray(spikes, dtype=np.float32)
    assert x.shape == (IN_FEATURES, K)
    assert weight.shape == (OUT_FEATURES, IN_FEATURES)

    nc = _get_bass()
    in_maps = _shard_inputs(x, weight, v, spikes)
    res = run_bass_kernel_spmd(
        nc, in_maps, core_ids=list(range(N_CORES)), trace=trace, **run_kwargs
    )
    out = np.empty((OUT_FEATURES, IN_FEATURES), dtype=np.float32)
    for j in range(N_CORES):
        out[:, j * SHARD:(j + 1) * SHARD] = res.results[j]["o"].astype(np.float32)
    return out, res


def kernel(x, weight, v, spikes, t=None, **_ignored):
    out, _ = run(x, weight, v, spikes, trace=False)
    return out
